# revision 16
# baseline (speedup 1.0000x reference)
"""Trainium2 Bass kernel for nn_LongTermMemory (retrieval_knn).

reference: best[b] = argmax_m cos(context[b], memory[m]); return
memory[best][None] -> [1, B, D].

Strategy (8 NeuronCores, memory sharded on M -> 8192 rows/core):
  Host prep: l2-normalize context and memory rows (cheap: 0.1% of FLOPs),
  cast to fp8-e4m3, and pack into the DoubleRow matmul layout
  (k = 256*t + 128*i + p).
  Device (per core): fp8 DoubleRow GEMM screening of all 512x8192 cosine
  sims, reduced on the fly into per-window statistics spread over all
  engines (three routes, mixed to balance engine load):
    - "A"    sim[b_low, m] tiles: DVE windowed max (32-row windows)
             straight from PSUM.
    - "Bold" simT[m_low, b] tiles: ACT exp-drain PSUM->SBUF bf16, Pool
             (gpsimd) partition-max (128-row windows of exp(sim)).
    - "B2"   simT[m_low, b] tiles: ACT exp-drain, then PE indicator-matmul
             window sums of exp (8-row windows), DVE drains the sums.
  exp is monotonic, so window exp-maxes / exp-sums upper-bound window
  maxes; the host converts via log/lambda + s0.
  Host: every window within MARGIN of a context's global screened max is
  re-ranked exactly in fp64 from the original fp32 inputs (fp8 screening
  only selects candidates; the final argmax is decided at fp64), then
  gather rows.
"""

import numpy as np
import ml_dtypes

import concourse.bacc as bacc
import concourse.tile as tile
from concourse import mybir
from concourse.bass_utils import run_bass_kernel_spmd

B, D, M_TOT = 512, 512, 65536
C = 8                      # cores
M = M_TOT // C             # 8192 rows per core
P = 128
NCH = 8                    # m-chunks of 1024 per core
CHM = M // NCH             # 1024
W = 32                     # A-window rows
NW_A = 512 // W            # 16 windows per A-group
WB2 = 8                    # B2-window rows
NW_B2 = P // WB2           # 16 windows per B2 m-tile
F32 = mybir.dt.float32
F16 = mybir.dt.float16
BF16 = mybir.dt.bfloat16
FP8 = mybir.dt.float8e4
DR = mybir.MatmulPerfMode.DoubleRow
E4M3 = ml_dtypes.float8_e4m3

LAM = 600.0                # exp sharpness
S0 = 0.2                   # exp offset: exp(LAM*(sim - S0))

# chunk compositions: two 512-m groups per 1024-m chunk.
CHUNKS = [("A", "Bold"), ("A", "B2"), ("Bold", "A"), ("B2", "Bold"),
          ("A", "B2"), ("Bold", "A"), ("B2", "Bold"), ("A", "Bold")]
N_A = sum(r.count("A") for r in CHUNKS)            # 7
N_B2 = sum(r.count("B2") for r in CHUNKS)          # 3
N_BOLD = sum(r.count("Bold") for r in CHUNKS)      # 6
N_BROW = 2 * N_BOLD                                # pool-op rows (per pair)

MARGIN = 0.02

_NC_CACHE = {}


def build_nc():
    key = "nc"
    if key in _NC_CACHE:
        return _NC_CACHE[key]
    from contextlib import ExitStack

    nc = bacc.Bacc("TRN2", target_bir_lowering=False, debug=False)
    ctx_dram = nc.dram_tensor("ctx8", [P, 2, 2, B], FP8, kind="ExternalInput")
    mem_dram = nc.dram_tensor("mem8", [NCH, P, 2, 2, CHM], FP8,
                              kind="ExternalInput")
    ind_dram = nc.dram_tensor("ind", [P, NW_B2], BF16, kind="ExternalInput")
    wa_dram = nc.dram_tensor("wmaxA", [P, N_A, 2, 2, NW_A], F16,
                             kind="ExternalOutput")
    pb_dram = nc.dram_tensor("pmaxB", [1, N_BROW, 2, B], BF16,
                             kind="ExternalOutput")
    ew_dram = nc.dram_tensor("expw", [N_B2, P, B], F32, kind="ExternalOutput")

    with tile.TileContext(nc) as tc, ExitStack() as ex:
        big = ex.enter_context(tc.tile_pool(name="big", bufs=1))
        mp = ex.enter_context(tc.tile_pool(name="mp", bufs=3))
        dr = ex.enter_context(tc.tile_pool(name="dr", bufs=5))
        ps = ex.enter_context(tc.tile_pool(name="ps", bufs=3, space="PSUM"))
        p2 = ex.enter_context(tc.tile_pool(name="p2", bufs=2, space="PSUM"))

        ctx8 = big.tile([P, 2, 2, B], FP8)
        nc.sync.dma_start(ctx8[:], ctx_dram[:])
        ind = big.tile([P, NW_B2], BF16)
        nc.sync.dma_start(ind[:], ind_dram[:])
        biast = big.tile([P, 1], F32)
        nc.vector.memset(biast[:], -LAM * S0)
        # pre-warm the Exp activation table during the initial DMA wait
        dummy = big.tile([P, 1], BF16)
        nc.scalar.activation(dummy[:], biast[:],
                             mybir.ActivationFunctionType.Exp,
                             scale=0.0, bias=biast[:])

        ia = ib = i2 = 0
        for ch in range(NCH):
            memsb = mp.tile([P, 2, 2, CHM], FP8, tag="memsb")
            if ch == 0:
                # split first load so compute starts after half arrives
                nc.sync.dma_start(memsb[:, :, :, 0:512],
                                  mem_dram[ch, :, :, :, 0:512])
                nc.sync.dma_start(memsb[:, :, :, 512:CHM],
                                  mem_dram[ch, :, :, :, 512:CHM])
            else:
                nc.sync.dma_start(memsb[:], mem_dram[ch])

            for gi, route in enumerate(CHUNKS[ch]):
                ms = gi * 512               # m offset within chunk
                if route == "A":
                    wA = dr.tile([P, 2, 2, NW_A], F16, tag="wA")
                    for pr in range(2):
                        acc = ps.tile([P, 2, 512], F32, tag="ps")
                        for hf in range(2):
                            bc = 2 * pr + hf
                            for t in range(2):
                                nc.tensor.matmul(
                                    acc[:, hf],
                                    ctx8[:, t, :, bc * P:(bc + 1) * P],
                                    memsb[:, t, :, ms:ms + 512],
                                    start=(t == 0), stop=(t == 1),
                                    perf_mode=DR)
                        nc.vector.tensor_reduce(
                            wA[:, pr],
                            acc[:].rearrange("p h (g w) -> p h g w", w=W),
                            axis=mybir.AxisListType.X,
                            op=mybir.AluOpType.max)
                    nc.sync.dma_start(wa_dram[:, ia], wA[:])
                    ia += 1
                elif route == "Bold":
                    for pr in range(2):
                        acc = ps.tile([P, 2, 512], F32, tag="ps")
                        for hf in range(2):
                            k0 = ms + (2 * pr + hf) * P
                            for t in range(2):
                                nc.tensor.matmul(
                                    acc[:, hf],
                                    memsb[:, t, :, k0:k0 + P],
                                    ctx8[:, t],
                                    start=(t == 0), stop=(t == 1),
                                    perf_mode=DR)
                        st = dr.tile([P, 2, 512], BF16, tag="drain")
                        nc.scalar.activation(
                            st[:], acc[:], mybir.ActivationFunctionType.Exp,
                            scale=LAM, bias=biast[:])
                        pB = dr.tile([1, 2, 512], BF16, tag="pB")
                        nc.gpsimd.tensor_reduce(
                            pB[:].rearrange("r h b -> r (h b)"),
                            st[:].rearrange("p h b -> p (h b)"),
                            axis=mybir.AxisListType.C,
                            op=mybir.AluOpType.max)
                        nc.sync.dma_start(pb_dram[:, ib], pB[:])
                        ib += 1
                else:  # B2
                    ps2t = p2.tile([P, 512], F32, tag="p2")
                    for pr in range(2):
                        acc = ps.tile([P, 2, 512], F32, tag="ps")
                        for hf in range(2):
                            k0 = ms + (2 * pr + hf) * P
                            for t in range(2):
                                nc.tensor.matmul(
                                    acc[:, hf],
                                    memsb[:, t, :, k0:k0 + P],
                                    ctx8[:, t],
                                    start=(t == 0), stop=(t == 1),
                                    perf_mode=DR)
                        ex8 = dr.tile([P, 2, 512], BF16, tag="exps")
                        nc.scalar.activation(
                            ex8[:], acc[:], mybir.ActivationFunctionType.Exp,
                            scale=LAM, bias=biast[:])
                        for hf in range(2):
                            j = 2 * pr + hf
                            nc.tensor.matmul(
                                ps2t[32 * j:32 * j + NW_B2, :],
                                ind[:], ex8[:, hf],
                                start=True, stop=True,
                                tile_position=(0, 32 * j))
                    exw = dr.tile([P, 512], F32, tag="exw")
                    nc.vector.tensor_copy(exw[:], ps2t[:])
                    nc.sync.dma_start(ew_dram[i2], exw[:])
                    i2 += 1

    nc.compile()
    _NC_CACHE[key] = nc
    return nc


def _pack_dr_T(xn8):
    """[N, 512 d] fp8 -> [p, t, i, N] DoubleRow layout (k = 256t+128i+p)."""
    n = xn8.shape[0]
    return np.ascontiguousarray(
        xn8.T.reshape(2, 2, P, n).transpose(2, 0, 1, 3))


def run_device(context, memory, trace=False):
    nc = build_nc()
    ctxn = context / np.sqrt(
        np.maximum((context.astype(np.float64) ** 2).sum(1, keepdims=True),
                   1e-12))
    memn = memory / np.sqrt(
        np.maximum((memory.astype(np.float64) ** 2).sum(1, keepdims=True),
                   1e-12))
    ctx8 = _pack_dr_T(ctxn.astype(E4M3))
    ind = (np.arange(P)[:, None] // WB2 ==
           np.arange(NW_B2)[None, :]).astype(ml_dtypes.bfloat16)
    in_maps = []
    for c in range(C):
        shard = memn[c * M:(c + 1) * M].astype(E4M3)
        arr = _pack_dr_T(shard)                       # [p, t, i, 8192]
        mem8 = np.ascontiguousarray(
            arr.reshape(P, 2, 2, NCH, CHM).transpose(3, 0, 1, 2, 4))
        in_maps.append({"ctx8": ctx8, "mem8": mem8, "ind": ind})
    return run_bass_kernel_spmd(nc, in_maps, list(range(C)), trace=trace)


def _window_tables():
    """Per-core windows as (m_start, m_len) in emission order per route."""
    a_starts, b_starts, e_starts = [], [], []
    for ch in range(NCH):
        for gi, route in enumerate(CHUNKS[ch]):
            ms = ch * CHM + gi * 512
            if route == "A":
                for w in range(NW_A):
                    a_starts.append(ms + w * W)
            elif route == "Bold":
                for k in range(4):        # pair-major: (pr, hf)
                    b_starts.append(ms + k * P)
            else:
                for j in range(4):
                    for w in range(NW_B2):
                        e_starts.append(ms + j * P + w * WB2)
    return (np.array(a_starts), np.array(b_starts), np.array(e_starts))


def kernel(context: np.ndarray, memory: np.ndarray) -> np.ndarray:
    res = run_device(context, memory)
    a_st, b_st, e_st = _window_tables()
    NA, NB, NE = len(a_st), len(b_st), len(e_st)   # per core
    NWC = NA + NB + NE

    vals = np.empty((B, C * NWC), dtype=np.float32)
    starts = np.empty(C * NWC, dtype=np.int64)
    lens = np.empty(C * NWC, dtype=np.int64)
    with np.errstate(divide="ignore"):
        for c in range(C):
            r = res.results[c]
            o = c * NWC
            # A: wmaxA [P, N_A, 2, 2, NW_A]; b = (2pr+hf)*128 + p
            va = r["wmaxA"].astype(np.float32)
            vals[:, o:o + NA] = va.transpose(2, 3, 0, 1, 4).reshape(B, NA)
            starts[o:o + NA] = c * M + a_st
            lens[o:o + NA] = W
            # Bold: pmaxB [1, N_BROW, 2, B] of exp -> log/lam + s0
            vb = r["pmaxB"].astype(np.float32).reshape(NB, B).T
            vals[:, o + NA:o + NA + NB] = np.log(vb) / LAM + S0
            starts[o + NA:o + NA + NB] = c * M + b_st
            lens[o + NA:o + NA + NB] = P
            # B2: expw [N_B2, P, B]: partition 32j+w -> (m-tile j, window w)
            ve = r["expw"].astype(np.float32)
            ve = ve.reshape(N_B2, 4, 32, B)[:, :, :NW_B2]   # [g2, j, w, B]
            vals[:, o + NA + NB:o + NWC] = (
                np.log(ve.reshape(NE, B)).T / LAM + S0)
            starts[o + NA + NB:o + NWC] = c * M + e_st
            lens[o + NA + NB:o + NWC] = WB2

    # exact fp64 re-rank of candidate windows
    ctx64 = context.astype(np.float64)
    mem64 = memory.astype(np.float64)
    ctxn = ctx64 / np.sqrt(np.maximum((ctx64 * ctx64).sum(1, keepdims=True),
                                      1e-12))
    mnorm = np.sqrt(np.maximum((mem64 * mem64).sum(1), 1e-12))
    gmax = vals.max(1)
    best = np.empty(B, dtype=np.int64)
    for b in range(B):
        sel = np.nonzero(vals[b] >= gmax[b] - MARGIN)[0]
        rows = np.concatenate(
            [starts[i] + np.arange(lens[i]) for i in sel])
        cos = (mem64[rows] @ ctxn[b]) / mnorm[rows]
        mx = cos.max()
        best[b] = rows[cos >= mx].min()
    return memory[best][None, :, :].astype(np.float32)


# revision 19
# speedup vs baseline: 1.0332x; 1.0332x over previous
"""Trainium2 Bass kernel for nn_LongTermMemory (retrieval_knn).

reference: best[b] = argmax_m cos(context[b], memory[m]); return
memory[best][None] -> [1, B, D].

Strategy (8 NeuronCores, memory sharded on M -> 8192 rows/core):
  Host prep: l2-normalize context and memory rows (cheap: 0.1% of FLOPs),
  cast to fp8-e4m3, and pack into the DoubleRow matmul layout
  (k = 256*t + 128*i + p).
  Device (per core): fp8 DoubleRow GEMM screening of all 512x8192 cosine
  sims, reduced on the fly into per-window statistics spread over all
  engines (three routes, mixed to balance engine load):
    - "A"    sim[b_low, m] tiles: DVE windowed max (32-row windows)
             straight from PSUM.
    - "Bold" simT[m_low, b] tiles: ACT exp-drain PSUM->SBUF bf16, Pool
             (gpsimd) partition-max (128-row windows of exp(sim)).
    - "B2"   simT[m_low, b] tiles: ACT exp-drain, then PE indicator-matmul
             window sums of exp (8-row windows), DVE drains the sums.
  exp is monotonic, so window exp-maxes / exp-sums upper-bound window
  maxes; the host converts via log/lambda + s0.
  Host: every window within MARGIN of a context's global screened max is
  re-ranked exactly in fp64 from the original fp32 inputs (fp8 screening
  only selects candidates; the final argmax is decided at fp64), then
  gather rows.
"""

import numpy as np
import ml_dtypes

import concourse.bacc as bacc
import concourse.tile as tile
from concourse import mybir
from concourse.bass_utils import run_bass_kernel_spmd

B, D, M_TOT = 512, 512, 65536
C = 8                      # cores
M = M_TOT // C             # 8192 rows per core
P = 128
NCH = 8                    # m-chunks of 1024 per core
CHM = M // NCH             # 1024
W = 32                     # A-window rows
NW_A = 512 // W            # 16 windows per A-group
WB2 = 8                    # B2-window rows
NW_B2 = P // WB2           # 16 windows per B2 m-tile
F32 = mybir.dt.float32
F16 = mybir.dt.float16
BF16 = mybir.dt.bfloat16
FP8 = mybir.dt.float8e4
DR = mybir.MatmulPerfMode.DoubleRow
E4M3 = ml_dtypes.float8_e4m3

LAM = 600.0                # exp sharpness
S0 = 0.2                   # exp offset: exp(LAM*(sim - S0))

# chunk compositions: two 512-m groups per 1024-m chunk.
CHUNKS = [("A", "Bold"), ("A", "B2"), ("Bold", "A"), ("B2", "Bold"),
          ("A", "B2"), ("Bold", "A"), ("B2", "Bold"), ("A", "Bold")]
N_A = sum(r.count("A") for r in CHUNKS)            # 7
N_B2 = sum(r.count("B2") for r in CHUNKS)          # 3
N_BOLD = sum(r.count("Bold") for r in CHUNKS)      # 6
N_BROW = 2 * N_BOLD                                # pool-op rows (per pair)

MARGIN = 0.02

_NC_CACHE = {}


def build_nc():
    key = "nc"
    if key in _NC_CACHE:
        return _NC_CACHE[key]
    from contextlib import ExitStack

    nc = bacc.Bacc("TRN2", target_bir_lowering=False, debug=False)
    ctx_dram = nc.dram_tensor("ctx8", [P, 2, 2, B], FP8, kind="ExternalInput")
    mem_dram = nc.dram_tensor("mem8", [NCH, P, 2, 2, CHM], FP8,
                              kind="ExternalInput")
    ind_dram = nc.dram_tensor("ind", [P, NW_B2], BF16, kind="ExternalInput")
    wa_dram = nc.dram_tensor("wmaxA", [P, N_A, 2, 2, NW_A], F16,
                             kind="ExternalOutput")
    pb_dram = nc.dram_tensor("pmaxB", [1, N_BROW, 2, B], BF16,
                             kind="ExternalOutput")
    ew_dram = nc.dram_tensor("expw", [N_B2, P, B], F32, kind="ExternalOutput")

    with tile.TileContext(nc) as tc, ExitStack() as ex:
        big = ex.enter_context(tc.tile_pool(name="big", bufs=1))
        mp = ex.enter_context(tc.tile_pool(name="mp", bufs=NCH))
        dr = ex.enter_context(tc.tile_pool(name="dr", bufs=5))
        psA = ex.enter_context(tc.tile_pool(name="psA", bufs=3, space="PSUM"))
        psB = ex.enter_context(tc.tile_pool(name="psB", bufs=4, space="PSUM"))
        p2 = ex.enter_context(tc.tile_pool(name="p2", bufs=1, space="PSUM"))

        ctx8 = big.tile([P, 2, 2, B], FP8)
        nc.sync.dma_start(ctx8[:], ctx_dram[:])
        ind = big.tile([P, NW_B2], BF16)
        nc.sync.dma_start(ind[:], ind_dram[:])
        biast = big.tile([P, 1], F32)
        nc.vector.memset(biast[:], -LAM * S0)
        # pre-warm the Exp activation table during the initial DMA wait
        dummy = big.tile([P, 1], BF16)
        nc.scalar.activation(dummy[:], biast[:],
                             mybir.ActivationFunctionType.Exp,
                             scale=0.0, bias=biast[:])

        ia = ib = i2 = 0
        chunks = []
        for ch in range(NCH):
            memsb = mp.tile([P, 2, 2, CHM], FP8, tag="memsb")
            if ch == 0:
                # split first load so compute starts after half arrives
                nc.sync.dma_start(memsb[:, :, :, 0:512],
                                  mem_dram[ch, :, :, :, 0:512])
                nc.sync.dma_start(memsb[:, :, :, 512:CHM],
                                  mem_dram[ch, :, :, :, 512:CHM])
            else:
                nc.sync.dma_start(memsb[:], mem_dram[ch])
            chunks.append(memsb)
        for ch in range(NCH):
            memsb = chunks[ch]
            for gi, route in enumerate(CHUNKS[ch]):
                ms = gi * 512               # m offset within chunk
                if route == "A":
                    wA = dr.tile([P, 2, 2, NW_A], F16, tag="wA")
                    for bc in range(4):
                        acc = psA.tile([P, 512], F32, tag="psA")
                        for t in range(2):
                            nc.tensor.matmul(
                                acc[:],
                                ctx8[:, t, :, bc * P:(bc + 1) * P],
                                memsb[:, t, :, ms:ms + 512],
                                start=(t == 0), stop=(t == 1),
                                perf_mode=DR)
                        nc.vector.tensor_reduce(
                            wA[:, bc // 2, bc % 2],
                            acc[:].rearrange("p (g w) -> p g w", w=W),
                            axis=mybir.AxisListType.X,
                            op=mybir.AluOpType.max)
                    nc.sync.dma_start(wa_dram[:, ia], wA[:])
                    ia += 1
                elif route == "Bold":
                    for pr in range(2):
                        st = dr.tile([P, 2, 512], BF16, tag="drain")
                        for hf in range(2):
                            acc = psB.tile([P, 512], F32, tag="psB")
                            k0 = ms + (2 * pr + hf) * P
                            for t in range(2):
                                nc.tensor.matmul(
                                    acc[:],
                                    memsb[:, t, :, k0:k0 + P],
                                    ctx8[:, t],
                                    start=(t == 0), stop=(t == 1),
                                    perf_mode=DR)
                            nc.scalar.activation(
                                st[:, hf], acc[:],
                                mybir.ActivationFunctionType.Exp,
                                scale=LAM, bias=biast[:])
                        pB = dr.tile([1, 2, 512], BF16, tag="pB")
                        nc.gpsimd.tensor_reduce(
                            pB[:].rearrange("r h b -> r (h b)"),
                            st[:].rearrange("p h b -> p (h b)"),
                            axis=mybir.AxisListType.C,
                            op=mybir.AluOpType.max)
                        nc.sync.dma_start(pb_dram[:, ib], pB[:])
                        ib += 1
                else:  # B2
                    ps2t = p2.tile([P, 512], F32, tag="p2")
                    for j in range(4):
                        acc = psB.tile([P, 512], F32, tag="psB")
                        k0 = ms + j * P
                        for t in range(2):
                            nc.tensor.matmul(
                                acc[:],
                                memsb[:, t, :, k0:k0 + P],
                                ctx8[:, t],
                                start=(t == 0), stop=(t == 1),
                                perf_mode=DR)
                        ex8 = dr.tile([P, 512], BF16, tag="exps")
                        nc.scalar.activation(
                            ex8[:], acc[:], mybir.ActivationFunctionType.Exp,
                            scale=LAM, bias=biast[:])
                        nc.tensor.matmul(
                            ps2t[32 * j:32 * j + NW_B2, :],
                            ind[:], ex8[:],
                            start=True, stop=True,
                            tile_position=(0, 32 * j))
                    exw = dr.tile([P, 512], F32, tag="exw")
                    nc.vector.tensor_copy(exw[:], ps2t[:])
                    nc.sync.dma_start(ew_dram[i2], exw[:])
                    i2 += 1

    nc.compile()
    _NC_CACHE[key] = nc
    return nc


def _pack_dr_T(xn8):
    """[N, 512 d] fp8 -> [p, t, i, N] DoubleRow layout (k = 256t+128i+p)."""
    n = xn8.shape[0]
    return np.ascontiguousarray(
        xn8.T.reshape(2, 2, P, n).transpose(2, 0, 1, 3))


def run_device(context, memory, trace=False):
    nc = build_nc()
    ctxn = context / np.sqrt(
        np.maximum((context.astype(np.float64) ** 2).sum(1, keepdims=True),
                   1e-12))
    memn = memory / np.sqrt(
        np.maximum((memory.astype(np.float64) ** 2).sum(1, keepdims=True),
                   1e-12))
    ctx8 = _pack_dr_T(ctxn.astype(E4M3))
    ind = (np.arange(P)[:, None] // WB2 ==
           np.arange(NW_B2)[None, :]).astype(ml_dtypes.bfloat16)
    in_maps = []
    for c in range(C):
        shard = memn[c * M:(c + 1) * M].astype(E4M3)
        arr = _pack_dr_T(shard)                       # [p, t, i, 8192]
        mem8 = np.ascontiguousarray(
            arr.reshape(P, 2, 2, NCH, CHM).transpose(3, 0, 1, 2, 4))
        in_maps.append({"ctx8": ctx8, "mem8": mem8, "ind": ind})
    return run_bass_kernel_spmd(nc, in_maps, list(range(C)), trace=trace)


def _window_tables():
    """Per-core windows as (m_start, m_len) in emission order per route."""
    a_starts, b_starts, e_starts = [], [], []
    for ch in range(NCH):
        for gi, route in enumerate(CHUNKS[ch]):
            ms = ch * CHM + gi * 512
            if route == "A":
                for w in range(NW_A):
                    a_starts.append(ms + w * W)
            elif route == "Bold":
                for k in range(4):        # pair-major: (pr, hf)
                    b_starts.append(ms + k * P)
            else:
                for j in range(4):
                    for w in range(NW_B2):
                        e_starts.append(ms + j * P + w * WB2)
    return (np.array(a_starts), np.array(b_starts), np.array(e_starts))


def kernel(context: np.ndarray, memory: np.ndarray) -> np.ndarray:
    res = run_device(context, memory)
    a_st, b_st, e_st = _window_tables()
    NA, NB, NE = len(a_st), len(b_st), len(e_st)   # per core
    NWC = NA + NB + NE

    vals = np.empty((B, C * NWC), dtype=np.float32)
    starts = np.empty(C * NWC, dtype=np.int64)
    lens = np.empty(C * NWC, dtype=np.int64)
    with np.errstate(divide="ignore"):
        for c in range(C):
            r = res.results[c]
            o = c * NWC
            # A: wmaxA [P, N_A, 2, 2, NW_A]; b = (2pr+hf)*128 + p
            va = r["wmaxA"].astype(np.float32)
            vals[:, o:o + NA] = va.transpose(2, 3, 0, 1, 4).reshape(B, NA)
            starts[o:o + NA] = c * M + a_st
            lens[o:o + NA] = W
            # Bold: pmaxB [1, N_BROW, 2, B] of exp -> log/lam + s0
            vb = r["pmaxB"].astype(np.float32).reshape(NB, B).T
            vals[:, o + NA:o + NA + NB] = np.log(vb) / LAM + S0
            starts[o + NA:o + NA + NB] = c * M + b_st
            lens[o + NA:o + NA + NB] = P
            # B2: expw [N_B2, P, B]: partition 32j+w -> (m-tile j, window w)
            ve = r["expw"].astype(np.float32)
            ve = ve.reshape(N_B2, 4, 32, B)[:, :, :NW_B2]   # [g2, j, w, B]
            vals[:, o + NA + NB:o + NWC] = (
                np.log(ve.reshape(NE, B)).T / LAM + S0)
            starts[o + NA + NB:o + NWC] = c * M + e_st
            lens[o + NA + NB:o + NWC] = WB2

    # exact fp64 re-rank of candidate windows
    ctx64 = context.astype(np.float64)
    mem64 = memory.astype(np.float64)
    ctxn = ctx64 / np.sqrt(np.maximum((ctx64 * ctx64).sum(1, keepdims=True),
                                      1e-12))
    mnorm = np.sqrt(np.maximum((mem64 * mem64).sum(1), 1e-12))
    gmax = vals.max(1)
    best = np.empty(B, dtype=np.int64)
    for b in range(B):
        sel = np.nonzero(vals[b] >= gmax[b] - MARGIN)[0]
        rows = np.concatenate(
            [starts[i] + np.arange(lens[i]) for i in sel])
        cos = (mem64[rows] @ ctxn[b]) / mnorm[rows]
        mx = cos.max()
        best[b] = rows[cos >= mx].min()
    return memory[best][None, :, :].astype(np.float32)
